# revision 4
# baseline (speedup 1.0000x reference)
"""TRN2 Bass kernel for CompressedLinearLayer: out = x @ (A @ B.T).T + bias.

Computed low-rank: t = x @ B  (rank 512), out = t @ A.T + bias.
Sharding: data-parallel over the 8192 rows of x (1024 rows per core);
B, A.T, bias replicated. No collectives.

Device layouts (per core):
  xT   [4096, 1024]  x rows shard, transposed on host (d_in on partitions)
  b    [4096, 512]   B as-is (d_in on partitions)
  at   [512, 4096]   A.T (rank on partitions)
  bias [4096]
  out  [1024, 4096]  natural orientation

Per core the 1024 rows are processed in 2 pipelined blocks of 512:
  stage1(b): tT[r, m] = sum_k B[k, r] * xT[k, m]   (rank on partitions)
  stage2(b): out[m, d] = sum_r tT[r, m] * AT[r, d] + bias[d]
stage2(b0) PE work overlaps stage1(b1) input DMA; output stores go out on
the scalar HWDGE ring while inputs stream on the sync ring.

Matmuls run in float32r (TF32-like, full PE rate at N>=256, rel err ~1.5e-4).
"""
import numpy as np

import concourse.bacc as bacc
import concourse.mybir as mybir
import concourse.tile as tile
from concourse.bass_utils import run_bass_kernel_spmd

N_CORES = 8
BATCH, SEQ = 4, 2048
D_IN, D_OUT, RANK = 4096, 4096, 512
ROWS_TOTAL = BATCH * SEQ           # 8192
ROWS = ROWS_TOTAL // N_CORES       # 1024 rows per core

F32 = mybir.dt.float32
F32R = mybir.dt.float32r

KC = D_IN // 128     # 32 contraction chunks, stage 1
RC = RANK // 128     # 4 rank chunks
NBLK = 2             # row blocks per core
BROWS = ROWS // NBLK                 # 512 rows per block
MB2 = BROWS // 128   # 4 row chunks of 128 per block (stage-2 out partitions)
DB2 = D_OUT // 512   # 8 d_out blocks of 512 (stage-2 moving dim)

_compiled = {}


def _build():
    nc = bacc.Bacc("TRN2", target_bir_lowering=False, debug=False)

    xT_d = nc.declare_dram_parameter("xT", [D_IN, ROWS], F32R, isOutput=False)
    b_d = nc.declare_dram_parameter("b", [D_IN, RANK], F32R, isOutput=False)
    at_d = nc.declare_dram_parameter("at", [RANK, D_OUT], F32R, isOutput=False)
    bias_d = nc.declare_dram_parameter("bias", [D_OUT], F32, isOutput=False)
    out_d = nc.declare_dram_parameter("out", [ROWS, D_OUT], F32, isOutput=True)

    with tile.TileContext(nc) as tc:
        with (
            tc.tile_pool(name="wb", bufs=1) as wb,
            tc.tile_pool(name="xp", bufs=6) as xp,
            tc.tile_pool(name="tt", bufs=1) as ttp,
            tc.tile_pool(name="op", bufs=2) as op,
            tc.tile_pool(name="ps1", bufs=4, space="PSUM") as ps1p,
            tc.tile_pool(name="ps2", bufs=4, space="PSUM") as ps2p,
        ):
            bias_bc = wb.tile([128, D_OUT], F32, tag="bias_bc")

            # B resident: 32 tiles [128, 512] (64KB/partition total)
            b_sb = [
                wb.tile([128, RANK], F32R, tag=f"b{k}", name=f"b{k}")
                for k in range(KC)
            ]
            # A.T resident: 4 tiles [128, 4096] (64KB/partition total)
            at_sb = [
                wb.tile([128, D_OUT], F32R, tag=f"at{r}", name=f"at{r}")
                for r in range(RC)
            ]
            # tT per block: 4 tiles [128, 512] f32r each
            tT = [
                [
                    ttp.tile([128, BROWS], F32R, tag=f"tT{b}_{r}", name=f"tT{b}_{r}")
                    for r in range(RC)
                ]
                for b in range(NBLK)
            ]

            def load_x(b, k):
                xk = xp.tile([128, BROWS], F32R, tag="xk", name=f"x{b}_{k}")
                nc.sync.dma_start(
                    xk[:], xT_d[k * 128:(k + 1) * 128, b * BROWS:(b + 1) * BROWS]
                )
                return xk

            def stage1(b):
                psum1 = [
                    ps1p.tile([128, BROWS], F32, tag="ps1", name=f"ps1_{b}_{i}")
                    for i in range(RC)
                ]
                for k in range(KC):
                    if b == 0:
                        # just-in-time interleave of B next to the x chunk that
                        # needs it on the sync DMA ring
                        nc.sync.dma_start(b_sb[k][:], b_d[k * 128:(k + 1) * 128, :])
                    xk = load_x(b, k)
                    for mc in range(RC):
                        nc.tensor.matmul(
                            psum1[mc][:],
                            b_sb[k][:, mc * 128:(mc + 1) * 128],
                            xk[:],
                            start=(k == 0),
                            stop=(k == KC - 1),
                        )
                for mc in range(RC):
                    nc.vector.tensor_copy(tT[b][mc][:], psum1[mc][:])

            def load_at():
                # on the scalar HWDGE ring (idle until the first out stores),
                # so it never queues behind B/x on the sync ring
                for r in range(RC):
                    nc.scalar.dma_start(at_sb[r][:], at_d[r * 128:(r + 1) * 128, :])

            def load_bias():
                nc.scalar.dma_start(bias_bc[0:1, :], bias_d[None, :])
                nc.gpsimd.partition_broadcast(bias_bc[:], bias_bc[0:1, :])

            def stage2(b):
                for rc2 in range(MB2):
                    row0 = rc2 * 128
                    for dch in range(DB2 // 4):     # two halves of d_out
                        psum2 = [
                            ps2p.tile(
                                [128, 512], F32, tag="ps2",
                                name=f"ps2_{b}_{rc2}_{dch}_{i}",
                            )
                            for i in range(4)
                        ]
                        for k in range(RC):
                            for dc in range(4):
                                d0 = (dch * 4 + dc) * 512
                                nc.tensor.matmul(
                                    psum2[dc][:],
                                    tT[b][k][:, row0:row0 + 128],
                                    at_sb[k][:, d0:d0 + 512],
                                    start=(k == 0),
                                    stop=(k == RC - 1),
                                )
                        ot = op.tile([128, 2048], F32, tag="ot", name=f"ot{b}_{rc2}_{dch}")
                        for dc in range(4):
                            d0 = (dch * 4 + dc) * 512
                            nc.vector.tensor_add(
                                ot[:, dc * 512:(dc + 1) * 512],
                                psum2[dc][:],
                                bias_bc[:, d0:d0 + 512],
                            )
                        # merged 1MB store on the scalar HWDGE ring
                        nc.scalar.dma_start(
                            out_d[
                                b * BROWS + row0:b * BROWS + row0 + 128,
                                dch * 2048:(dch + 1) * 2048,
                            ],
                            ot[:],
                        )

            load_at()
            load_bias()
            stage1(0)
            stage2(0)
            stage1(1)
            stage2(1)

    nc.compile()
    return nc


def _get_nc():
    if "nc" not in _compiled:
        _compiled["nc"] = _build()
    return _compiled["nc"]


def run(inputs, trace=False, trace_kwargs=None):
    """Shard, execute on 8 cores, gather. Returns (output, BassKernelResults)."""
    x = np.asarray(inputs["x"], dtype=np.float32)
    A = np.asarray(inputs["A"], dtype=np.float32)
    B = np.asarray(inputs["B"], dtype=np.float32)
    bias = np.asarray(inputs["bias"], dtype=np.float32)

    x_flat = x.reshape(ROWS_TOTAL, D_IN)
    AT = np.ascontiguousarray(A.T)
    in_maps = []
    for i in range(N_CORES):
        xT_i = np.ascontiguousarray(x_flat[i * ROWS:(i + 1) * ROWS].T)
        in_maps.append({"xT": xT_i, "b": B, "at": AT, "bias": bias})

    nc = _get_nc()
    kwargs = {}
    if trace:
        kwargs["trace"] = True
        kwargs["trace_kwargs"] = trace_kwargs or {}
    res = run_bass_kernel_spmd(nc, in_maps, core_ids=list(range(N_CORES)), **kwargs)

    out = np.concatenate([res.results[i]["out"] for i in range(N_CORES)], axis=0)
    return out.reshape(BATCH, SEQ, D_OUT), res


def kernel(**inputs) -> np.ndarray:
    out, _ = run(inputs)
    return out


# revision 5
# speedup vs baseline: 1.0990x; 1.0990x over previous
"""TRN2 Bass kernel for CompressedLinearLayer: out = x @ (A @ B.T).T + bias.

Computed low-rank: t = x @ B  (rank 512), out = t @ A.T + bias.
Sharding: data-parallel over the 8192 rows of x (1024 rows per core);
B, A.T, bias replicated. No collectives.

Device layouts (per core):
  xT   [4096, 1024]  x rows shard, transposed on host (d_in on partitions)
  b    [4096, 512]   B as-is (d_in on partitions)
  at   [512, 4096]   A.T (rank on partitions)
  bias [4096]
  out  [1024, 4096]  natural orientation

Per core the 1024 rows are processed in 2 pipelined blocks of 512:
  stage1(b): tT[r, m] = sum_k B[k, r] * xT[k, m]   (rank on partitions)
  stage2(b): out[m, d] = sum_r tT[r, m] * AT[r, d] + bias[d]
Inputs stream on the sync HWDGE ring in 1MB chunks (4 contraction
sub-chunks per dma_start to amortize the ~0.8us per-issue sequencer cost);
A.T/bias/output stores use the scalar HWDGE ring.

Matmuls run in float32r (~1.8 cyc/col on HW, rel err ~1.5e-4).
"""
import numpy as np

import concourse.bacc as bacc
import concourse.mybir as mybir
import concourse.tile as tile
from concourse.bass_utils import run_bass_kernel_spmd

N_CORES = 8
BATCH, SEQ = 4, 2048
D_IN, D_OUT, RANK = 4096, 4096, 512
ROWS_TOTAL = BATCH * SEQ           # 8192
ROWS = ROWS_TOTAL // N_CORES       # 1024 rows per core

F32 = mybir.dt.float32
F32R = mybir.dt.float32r

KC = D_IN // 128     # 32 contraction chunks, stage 1
KSUB = 4             # k-chunks packed per DMA (1MB transfers)
KB = KC // KSUB      # 8 packed k-groups
RC = RANK // 128     # 4 rank chunks
NBLK = 2             # row blocks per core
BROWS = ROWS // NBLK                 # 512 rows per block
MB2 = BROWS // 128   # 4 row chunks of 128 per block (stage-2 out partitions)
DB2 = D_OUT // 512   # 8 d_out blocks of 512 (stage-2 moving dim)

_compiled = {}


def _build():
    nc = bacc.Bacc("TRN2", target_bir_lowering=False, debug=False)

    xT_d = nc.declare_dram_parameter("xT", [D_IN, ROWS], F32R, isOutput=False)
    b_d = nc.declare_dram_parameter("b", [D_IN, RANK], F32R, isOutput=False)
    at_d = nc.declare_dram_parameter("at", [RANK, D_OUT], F32R, isOutput=False)
    bias_d = nc.declare_dram_parameter("bias", [D_OUT], F32, isOutput=False)
    out_d = nc.declare_dram_parameter("out", [ROWS, D_OUT], F32, isOutput=True)

    with tile.TileContext(nc) as tc:
        with (
            tc.tile_pool(name="wb", bufs=1) as wb,
            tc.tile_pool(name="xp", bufs=3) as xp,
            tc.tile_pool(name="tt", bufs=1) as ttp,
            tc.tile_pool(name="op", bufs=2) as op,
            tc.tile_pool(name="ps1", bufs=4, space="PSUM") as ps1p,
            tc.tile_pool(name="ps2", bufs=4, space="PSUM") as ps2p,
        ):
            bias_bc = wb.tile([128, D_OUT], F32, tag="bias_bc")

            # B resident: 8 tiles [128, 4, 512] = 1MB each (64KB/partition total)
            b_sb = [
                wb.tile([128, KSUB, RANK], F32R, tag=f"b{g}", name=f"b{g}")
                for g in range(KB)
            ]
            # A.T resident: 4 tiles [128, 4096] (64KB/partition total)
            at_sb = [
                wb.tile([128, D_OUT], F32R, tag=f"at{r}", name=f"at{r}")
                for r in range(RC)
            ]
            # tT per block: 4 tiles [128, 512] f32r each
            tT = [
                [
                    ttp.tile([128, BROWS], F32R, tag=f"tT{b}_{r}", name=f"tT{b}_{r}")
                    for r in range(RC)
                ]
                for b in range(NBLK)
            ]

            def stage1(b):
                psum1 = [
                    ps1p.tile([128, BROWS], F32, tag="ps1", name=f"ps1_{b}_{i}")
                    for i in range(RC)
                ]
                for g in range(KB):
                    if b == 0:
                        # just-in-time interleave of B next to the x group that
                        # needs it on the sync DMA ring (1MB each)
                        nc.sync.dma_start(
                            b_sb[g][:],
                            b_d[g * KSUB * 128:(g + 1) * KSUB * 128, :].rearrange(
                                "(ks p) r -> p ks r", p=128
                            ),
                        )
                    xg = xp.tile([128, KSUB, BROWS], F32R, tag="xk", name=f"x{b}_{g}")
                    nc.sync.dma_start(
                        xg[:],
                        xT_d[
                            g * KSUB * 128:(g + 1) * KSUB * 128,
                            b * BROWS:(b + 1) * BROWS,
                        ].rearrange("(ks p) m -> p ks m", p=128),
                    )
                    for ks in range(KSUB):
                        k = g * KSUB + ks
                        for mc in range(RC):
                            nc.tensor.matmul(
                                psum1[mc][:],
                                b_sb[g][:, ks, mc * 128:(mc + 1) * 128],
                                xg[:, ks, :],
                                start=(k == 0),
                                stop=(k == KC - 1),
                            )
                for mc in range(RC):
                    nc.vector.tensor_copy(tT[b][mc][:], psum1[mc][:])

            def load_at():
                # on the scalar HWDGE ring (idle until the first out stores),
                # so it never queues behind B/x on the sync ring
                for r in range(RC):
                    nc.scalar.dma_start(at_sb[r][:], at_d[r * 128:(r + 1) * 128, :])

            def load_bias():
                nc.scalar.dma_start(bias_bc[0:1, :], bias_d[None, :])
                nc.gpsimd.partition_broadcast(bias_bc[:], bias_bc[0:1, :])

            def stage2(b):
                for rc2 in range(MB2):
                    row0 = rc2 * 128
                    for dch in range(DB2 // 4):     # two halves of d_out
                        psum2 = [
                            ps2p.tile(
                                [128, 512], F32, tag="ps2",
                                name=f"ps2_{b}_{rc2}_{dch}_{i}",
                            )
                            for i in range(4)
                        ]
                        for k in range(RC):
                            for dc in range(4):
                                d0 = (dch * 4 + dc) * 512
                                nc.tensor.matmul(
                                    psum2[dc][:],
                                    tT[b][k][:, row0:row0 + 128],
                                    at_sb[k][:, d0:d0 + 512],
                                    start=(k == 0),
                                    stop=(k == RC - 1),
                                )
                        ot = op.tile([128, 2048], F32, tag="ot", name=f"ot{b}_{rc2}_{dch}")
                        for dc in range(4):
                            d0 = (dch * 4 + dc) * 512
                            nc.vector.tensor_add(
                                ot[:, dc * 512:(dc + 1) * 512],
                                psum2[dc][:],
                                bias_bc[:, d0:d0 + 512],
                            )
                        # merged 1MB store on the scalar HWDGE ring
                        nc.scalar.dma_start(
                            out_d[
                                b * BROWS + row0:b * BROWS + row0 + 128,
                                dch * 2048:(dch + 1) * 2048,
                            ],
                            ot[:],
                        )

            load_at()
            load_bias()
            stage1(0)
            stage2(0)
            stage1(1)
            stage2(1)

    nc.compile()
    return nc


def _get_nc():
    if "nc" not in _compiled:
        _compiled["nc"] = _build()
    return _compiled["nc"]


def run(inputs, trace=False, trace_kwargs=None):
    """Shard, execute on 8 cores, gather. Returns (output, BassKernelResults)."""
    x = np.asarray(inputs["x"], dtype=np.float32)
    A = np.asarray(inputs["A"], dtype=np.float32)
    B = np.asarray(inputs["B"], dtype=np.float32)
    bias = np.asarray(inputs["bias"], dtype=np.float32)

    x_flat = x.reshape(ROWS_TOTAL, D_IN)
    AT = np.ascontiguousarray(A.T)
    in_maps = []
    for i in range(N_CORES):
        xT_i = np.ascontiguousarray(x_flat[i * ROWS:(i + 1) * ROWS].T)
        in_maps.append({"xT": xT_i, "b": B, "at": AT, "bias": bias})

    nc = _get_nc()
    kwargs = {}
    if trace:
        kwargs["trace"] = True
        kwargs["trace_kwargs"] = trace_kwargs or {}
    res = run_bass_kernel_spmd(nc, in_maps, core_ids=list(range(N_CORES)), **kwargs)

    out = np.concatenate([res.results[i]["out"] for i in range(N_CORES)], axis=0)
    return out.reshape(BATCH, SEQ, D_OUT), res


def kernel(**inputs) -> np.ndarray:
    out, _ = run(inputs)
    return out


# revision 7
# speedup vs baseline: 1.3111x; 1.1930x over previous
"""TRN2 Bass kernel for CompressedLinearLayer: out = x @ (A @ B.T).T + bias.

Computed low-rank: t = x @ B  (rank 512), out = t @ A.T + bias.
Sharding: data-parallel over the 8192 rows of x (1024 rows per core);
B, A.T, bias replicated. No collectives.

Device layouts (per core), bf16 on the wire for matmul inputs:
  xT   [4096, 1024] bf16  x rows shard, transposed+converted on host
  b    [4096, 512]  bf16  B
  at   [512, 4096]  bf16  A.T
  bias [4096]       f32
  out  [1024, 4096] f32   natural orientation

Per core the 1024 rows are processed in 2 blocks of 512:
  stage1(b): tT[r, m] = sum_k B[k, r] * xT[k, m]   (rank on partitions)
  stage2(b): out[m, d] = sum_r tT[r, m] * AT[r, d] + bias[d]
stage2(0) units are interleaved with stage1(1) groups so the PE never
starves while block-1 x chunks stream in. Inputs stream on the sync
HWDGE ring in 1MB chunks; A.T (ordering-delayed), bias and the output
stores ride the scalar HWDGE ring. Accumulation is fp32 in PSUM.
"""
import numpy as np
import ml_dtypes

import concourse.bacc as bacc
import concourse.mybir as mybir
import concourse.tile as tile
from concourse.tile import add_dep_helper
from concourse.bass_utils import run_bass_kernel_spmd

N_CORES = 8
BATCH, SEQ = 4, 2048
D_IN, D_OUT, RANK = 4096, 4096, 512
ROWS_TOTAL = BATCH * SEQ           # 8192
ROWS = ROWS_TOTAL // N_CORES       # 1024 rows per core

F32 = mybir.dt.float32
BF16 = mybir.dt.bfloat16

KC = D_IN // 128     # 32 contraction chunks, stage 1
KSUB = 8             # k-chunks packed per DMA (1MB bf16 transfers)
KB = KC // KSUB      # 4 packed k-groups
RC = RANK // 128     # 4 rank chunks
NBLK = 2             # row blocks per core
BROWS = ROWS // NBLK                 # 512 rows per block
MB2 = BROWS // 128   # 4 row chunks of 128 per block (stage-2 out partitions)
DB2 = D_OUT // 512   # 8 d_out blocks of 512 (stage-2 moving dim)

_compiled = {}


def _build():
    nc = bacc.Bacc("TRN2", target_bir_lowering=False, debug=False)

    xT_d = nc.declare_dram_parameter("xT", [D_IN, ROWS], BF16, isOutput=False)
    b_d = nc.declare_dram_parameter("b", [D_IN, RANK], BF16, isOutput=False)
    at_d = nc.declare_dram_parameter("at", [RANK, D_OUT], BF16, isOutput=False)
    bias_d = nc.declare_dram_parameter("bias", [D_OUT], F32, isOutput=False)
    out_d = nc.declare_dram_parameter("out", [ROWS, D_OUT], F32, isOutput=True)

    with tile.TileContext(nc) as tc:
        with (
            tc.tile_pool(name="wb", bufs=1) as wb,
            tc.tile_pool(name="xp", bufs=4) as xp,
            tc.tile_pool(name="tt", bufs=1) as ttp,
            tc.tile_pool(name="op", bufs=3) as op,
            tc.tile_pool(name="ps1", bufs=4, space="PSUM") as ps1p,
            tc.tile_pool(name="ps2", bufs=4, space="PSUM") as ps2p,
        ):
            bias_bc = wb.tile([128, D_OUT], F32, tag="bias_bc")

            # B resident: 4 tiles [128, 8, 512] bf16 = 1MB each
            b_sb = [
                wb.tile([128, KSUB, RANK], BF16, tag=f"b{g}", name=f"b{g}")
                for g in range(KB)
            ]
            # A.T resident: 4 tiles [128, 4096] bf16 (1MB each)
            at_sb = [
                wb.tile([128, D_OUT], BF16, tag=f"at{r}", name=f"at{r}")
                for r in range(RC)
            ]
            # tT per block: 4 tiles [128, 512] bf16 each
            tT = [
                [
                    ttp.tile([128, BROWS], BF16, tag=f"tT{b}_{r}", name=f"tT{b}_{r}")
                    for r in range(RC)
                ]
                for b in range(NBLK)
            ]

            x_dmas = {}

            def stage1_group(b, g, psum1):
                if b == 0:
                    nc.sync.dma_start(
                        b_sb[g][:],
                        b_d[g * KSUB * 128:(g + 1) * KSUB * 128, :].rearrange(
                            "(ks p) r -> p ks r", p=128
                        ),
                    )
                xg = xp.tile([128, KSUB, BROWS], BF16, tag="xk", name=f"x{b}_{g}")
                x_dmas[(b, g)] = nc.sync.dma_start(
                    xg[:],
                    xT_d[
                        g * KSUB * 128:(g + 1) * KSUB * 128,
                        b * BROWS:(b + 1) * BROWS,
                    ].rearrange("(ks p) m -> p ks m", p=128),
                )
                for ks in range(KSUB):
                    k = g * KSUB + ks
                    for mc in range(RC):
                        nc.tensor.matmul(
                            psum1[mc][:],
                            b_sb[g][:, ks, mc * 128:(mc + 1) * 128],
                            xg[:, ks, :],
                            start=(k == 0),
                            stop=(k == KC - 1),
                        )

            def stage1_psum(b):
                return [
                    ps1p.tile([128, BROWS], F32, tag="ps1", name=f"ps1_{b}_{i}")
                    for i in range(RC)
                ]

            def stage1_evac(b, psum1):
                for mc in range(RC):
                    nc.vector.tensor_copy(tT[b][mc][:], psum1[mc][:])

            def load_at(after_dma):
                # scalar HWDGE ring, ordering-delayed so it doesn't starve
                # the block-0 x stream of HBM bandwidth
                for r in range(RC):
                    at_dma = nc.scalar.dma_start(
                        at_sb[r][:], at_d[r * 128:(r + 1) * 128, :]
                    )
                    if after_dma is not None:
                        add_dep_helper(
                            at_dma.ins,
                            after_dma.ins,
                            sync=True,
                            reason="delay A.T load behind block-0 x stream",
                        )

            def load_bias():
                nc.scalar.dma_start(bias_bc[0:1, :], bias_d[None, :])
                nc.gpsimd.partition_broadcast(bias_bc[:], bias_bc[0:1, :])

            def stage2_unit(b, rc2, dch):
                row0 = rc2 * 128
                psum2 = [
                    ps2p.tile(
                        [128, 512], F32, tag="ps2",
                        name=f"ps2_{b}_{rc2}_{dch}_{i}",
                    )
                    for i in range(4)
                ]
                for k in range(RC):
                    for dc in range(4):
                        d0 = (dch * 4 + dc) * 512
                        nc.tensor.matmul(
                            psum2[dc][:],
                            tT[b][k][:, row0:row0 + 128],
                            at_sb[k][:, d0:d0 + 512],
                            start=(k == 0),
                            stop=(k == RC - 1),
                        )
                ot = op.tile([128, 2048], F32, tag="ot", name=f"ot{b}_{rc2}_{dch}")
                for dc in range(4):
                    d0 = (dch * 4 + dc) * 512
                    nc.vector.tensor_add(
                        ot[:, dc * 512:(dc + 1) * 512],
                        psum2[dc][:],
                        bias_bc[:, d0:d0 + 512],
                    )
                nc.scalar.dma_start(
                    out_d[
                        b * BROWS + row0:b * BROWS + row0 + 128,
                        dch * 2048:(dch + 1) * 2048,
                    ],
                    ot[:],
                )

            load_bias()

            # stage1 block 0
            ps_a = stage1_psum(0)
            for g in range(KB):
                stage1_group(0, g, ps_a)
                if g == 1:
                    load_at(x_dmas[(0, 1)])
            stage1_evac(0, ps_a)

            # interleave stage2(0) units with stage1(1) groups
            ps_b = stage1_psum(1)
            units = [(0, rc2, dch) for rc2 in range(MB2) for dch in range(DB2 // 4)]
            gi = 0
            for i, u in enumerate(units):
                stage2_unit(*u)
                if i % 2 == 0 and gi < KB:
                    stage1_group(1, gi, ps_b)
                    gi += 1
            while gi < KB:
                stage1_group(1, gi, ps_b)
                gi += 1
            stage1_evac(1, ps_b)

            for rc2 in range(MB2):
                for dch in range(DB2 // 4):
                    stage2_unit(1, rc2, dch)

    nc.compile()
    return nc


def _get_nc():
    if "nc" not in _compiled:
        _compiled["nc"] = _build()
    return _compiled["nc"]


def run(inputs, trace=False, trace_kwargs=None):
    """Shard, execute on 8 cores, gather. Returns (output, BassKernelResults)."""
    x = np.asarray(inputs["x"], dtype=np.float32)
    A = np.asarray(inputs["A"], dtype=np.float32)
    B = np.asarray(inputs["B"], dtype=np.float32)
    bias = np.asarray(inputs["bias"], dtype=np.float32)

    x_flat = x.reshape(ROWS_TOTAL, D_IN)
    B_bf = B.astype(ml_dtypes.bfloat16)
    AT_bf = np.ascontiguousarray(A.T).astype(ml_dtypes.bfloat16)
    in_maps = []
    for i in range(N_CORES):
        xT_i = np.ascontiguousarray(x_flat[i * ROWS:(i + 1) * ROWS].T).astype(
            ml_dtypes.bfloat16
        )
        in_maps.append({"xT": xT_i, "b": B_bf, "at": AT_bf, "bias": bias})

    nc = _get_nc()
    kwargs = {}
    if trace:
        kwargs["trace"] = True
        kwargs["trace_kwargs"] = trace_kwargs or {}
    res = run_bass_kernel_spmd(nc, in_maps, core_ids=list(range(N_CORES)), **kwargs)

    out = np.concatenate([res.results[i]["out"] for i in range(N_CORES)], axis=0)
    return out.reshape(BATCH, SEQ, D_OUT), res


def kernel(**inputs) -> np.ndarray:
    out, _ = run(inputs)
    return out


# revision 8
# speedup vs baseline: 1.3254x; 1.0109x over previous
"""TRN2 Bass kernel for CompressedLinearLayer: out = x @ (A @ B.T).T + bias.

Computed low-rank: t = x @ B  (rank 512), out = t @ A.T + bias.
Sharding: data-parallel over the 8192 rows of x (1024 rows per core);
B, A.T, bias replicated. No collectives.

Device layouts (per core), bf16 on the wire for matmul inputs:
  xT   [4096, 1024] bf16  x rows shard, transposed+converted on host
  b    [4096, 512]  bf16  B
  at   [512, 4096]  bf16  A.T
  bias [4096]       f32
  out  [1024, 4096] f32   natural orientation

Per core the 1024 rows are processed in 2 blocks of 512:
  stage1(b): tT[r, m] = sum_k B[k, r] * xT[k, m]   (rank on partitions)
  stage2(b): out[m, d] = sum_r tT[r, m] * AT[r, d] + bias[d]
stage2(0) units are interleaved with stage1(1) groups so the PE never
starves while block-1 x chunks stream in. Inputs stream on the sync
HWDGE ring in 1MB chunks; A.T (ordering-delayed), bias and the output
stores ride the scalar HWDGE ring. Accumulation is fp32 in PSUM.
"""
import numpy as np
import ml_dtypes

import concourse.bacc as bacc
import concourse.mybir as mybir
import concourse.tile as tile
from concourse.tile import add_dep_helper
from concourse.bass_utils import run_bass_kernel_spmd

N_CORES = 8
BATCH, SEQ = 4, 2048
D_IN, D_OUT, RANK = 4096, 4096, 512
ROWS_TOTAL = BATCH * SEQ           # 8192
ROWS = ROWS_TOTAL // N_CORES       # 1024 rows per core

F32 = mybir.dt.float32
BF16 = mybir.dt.bfloat16

KC = D_IN // 128     # 32 contraction chunks, stage 1
KSUB = 4             # k-chunks packed per DMA (0.5MB bf16 transfers)
KB = KC // KSUB      # 4 packed k-groups
RC = RANK // 128     # 4 rank chunks
NBLK = 2             # row blocks per core
BROWS = ROWS // NBLK                 # 512 rows per block
MB2 = BROWS // 128   # 4 row chunks of 128 per block (stage-2 out partitions)
DB2 = D_OUT // 512   # 8 d_out blocks of 512 (stage-2 moving dim)

_compiled = {}


def _build():
    nc = bacc.Bacc("TRN2", target_bir_lowering=False, debug=False)

    xT_d = nc.declare_dram_parameter("xT", [D_IN, ROWS], BF16, isOutput=False)
    b_d = nc.declare_dram_parameter("b", [D_IN, RANK], BF16, isOutput=False)
    at_d = nc.declare_dram_parameter("at", [RANK, D_OUT], BF16, isOutput=False)
    bias_d = nc.declare_dram_parameter("bias", [D_OUT], F32, isOutput=False)
    out_d = nc.declare_dram_parameter("out", [ROWS, D_OUT], F32, isOutput=True)

    with tile.TileContext(nc) as tc:
        with (
            tc.tile_pool(name="wb", bufs=1) as wb,
            tc.tile_pool(name="xp", bufs=4) as xp,
            tc.tile_pool(name="tt", bufs=1) as ttp,
            tc.tile_pool(name="op", bufs=3) as op,
            tc.tile_pool(name="ps1", bufs=4, space="PSUM") as ps1p,
            tc.tile_pool(name="ps2", bufs=4, space="PSUM") as ps2p,
        ):
            bias_bc = wb.tile([128, D_OUT], F32, tag="bias_bc")

            # B resident: 8 tiles [128, 4, 512] bf16 = 0.5MB each
            b_sb = [
                wb.tile([128, KSUB, RANK], BF16, tag=f"b{g}", name=f"b{g}")
                for g in range(KB)
            ]
            # A.T resident: 4 tiles [128, 4096] bf16 (1MB each)
            at_sb = [
                wb.tile([128, D_OUT], BF16, tag=f"at{r}", name=f"at{r}")
                for r in range(RC)
            ]
            # tT per block: 4 tiles [128, 512] bf16 each
            tT = [
                [
                    ttp.tile([128, BROWS], BF16, tag=f"tT{b}_{r}", name=f"tT{b}_{r}")
                    for r in range(RC)
                ]
                for b in range(NBLK)
            ]

            x_dmas = {}

            def stage1_group(b, g, psum1):
                if b == 0:
                    nc.sync.dma_start(
                        b_sb[g][:],
                        b_d[g * KSUB * 128:(g + 1) * KSUB * 128, :].rearrange(
                            "(ks p) r -> p ks r", p=128
                        ),
                    )
                xg = xp.tile([128, KSUB, BROWS], BF16, tag="xk", name=f"x{b}_{g}")
                x_dmas[(b, g)] = nc.sync.dma_start(
                    xg[:],
                    xT_d[
                        g * KSUB * 128:(g + 1) * KSUB * 128,
                        b * BROWS:(b + 1) * BROWS,
                    ].rearrange("(ks p) m -> p ks m", p=128),
                )
                last = g == KB - 1
                if not last:
                    for ks in range(KSUB):
                        k = g * KSUB + ks
                        for mc in range(RC):
                            nc.tensor.matmul(
                                psum1[mc][:],
                                b_sb[g][:, ks, mc * 128:(mc + 1) * 128],
                                xg[:, ks, :],
                                start=(k == 0),
                                stop=False,
                            )
                else:
                    # invert loops so each psum finishes (and can evacuate to
                    # tT on the DVE) while the PE continues with the next mc
                    for mc in range(RC):
                        for ks in range(KSUB):
                            k = g * KSUB + ks
                            nc.tensor.matmul(
                                psum1[mc][:],
                                b_sb[g][:, ks, mc * 128:(mc + 1) * 128],
                                xg[:, ks, :],
                                start=False,
                                stop=(ks == KSUB - 1),
                            )
                        nc.vector.tensor_copy(tT[b][mc][:], psum1[mc][:])

            def stage1_psum(b):
                return [
                    ps1p.tile([128, BROWS], F32, tag="ps1", name=f"ps1_{b}_{i}")
                    for i in range(RC)
                ]

            def load_at(after_dma):
                # scalar HWDGE ring, ordering-delayed so it doesn't starve
                # the block-0 x stream of HBM bandwidth
                for r in range(RC):
                    at_dma = nc.scalar.dma_start(
                        at_sb[r][:], at_d[r * 128:(r + 1) * 128, :]
                    )
                    if after_dma is not None:
                        add_dep_helper(
                            at_dma.ins,
                            after_dma.ins,
                            sync=True,
                            reason="delay A.T load behind block-0 x stream",
                        )

            def load_bias():
                nc.scalar.dma_start(bias_bc[0:1, :], bias_d[None, :])
                nc.gpsimd.partition_broadcast(bias_bc[:], bias_bc[0:1, :])

            def stage2_unit(b, rc2, dch):
                row0 = rc2 * 128
                psum2 = [
                    ps2p.tile(
                        [128, 512], F32, tag="ps2",
                        name=f"ps2_{b}_{rc2}_{dch}_{i}",
                    )
                    for i in range(4)
                ]
                for k in range(RC):
                    for dc in range(4):
                        d0 = (dch * 4 + dc) * 512
                        nc.tensor.matmul(
                            psum2[dc][:],
                            tT[b][k][:, row0:row0 + 128],
                            at_sb[k][:, d0:d0 + 512],
                            start=(k == 0),
                            stop=(k == RC - 1),
                        )
                ot = op.tile([128, 2048], F32, tag="ot", name=f"ot{b}_{rc2}_{dch}")
                for dc in range(4):
                    d0 = (dch * 4 + dc) * 512
                    nc.vector.tensor_add(
                        ot[:, dc * 512:(dc + 1) * 512],
                        psum2[dc][:],
                        bias_bc[:, d0:d0 + 512],
                    )
                nc.scalar.dma_start(
                    out_d[
                        b * BROWS + row0:b * BROWS + row0 + 128,
                        dch * 2048:(dch + 1) * 2048,
                    ],
                    ot[:],
                )

            load_bias()

            # stage1 block 0
            ps_a = stage1_psum(0)
            for g in range(KB):
                stage1_group(0, g, ps_a)
                if g == 1:
                    load_at(x_dmas[(0, 1)])

            # interleave stage2(0) units with stage1(1) groups
            ps_b = stage1_psum(1)
            units = [(0, rc2, dch) for rc2 in range(MB2) for dch in range(DB2 // 4)]
            gi = 0
            for i, u in enumerate(units):
                stage2_unit(*u)
                if i % 2 == 0 and gi < KB:
                    stage1_group(1, gi, ps_b)
                    gi += 1
            while gi < KB:
                stage1_group(1, gi, ps_b)
                gi += 1

            for rc2 in range(MB2):
                for dch in range(DB2 // 4):
                    stage2_unit(1, rc2, dch)

    nc.compile()
    return nc


def _get_nc():
    if "nc" not in _compiled:
        _compiled["nc"] = _build()
    return _compiled["nc"]


def run(inputs, trace=False, trace_kwargs=None):
    """Shard, execute on 8 cores, gather. Returns (output, BassKernelResults)."""
    x = np.asarray(inputs["x"], dtype=np.float32)
    A = np.asarray(inputs["A"], dtype=np.float32)
    B = np.asarray(inputs["B"], dtype=np.float32)
    bias = np.asarray(inputs["bias"], dtype=np.float32)

    x_flat = x.reshape(ROWS_TOTAL, D_IN)
    B_bf = B.astype(ml_dtypes.bfloat16)
    AT_bf = np.ascontiguousarray(A.T).astype(ml_dtypes.bfloat16)
    in_maps = []
    for i in range(N_CORES):
        xT_i = np.ascontiguousarray(x_flat[i * ROWS:(i + 1) * ROWS].T).astype(
            ml_dtypes.bfloat16
        )
        in_maps.append({"xT": xT_i, "b": B_bf, "at": AT_bf, "bias": bias})

    nc = _get_nc()
    kwargs = {}
    if trace:
        kwargs["trace"] = True
        kwargs["trace_kwargs"] = trace_kwargs or {}
    res = run_bass_kernel_spmd(nc, in_maps, core_ids=list(range(N_CORES)), **kwargs)

    out = np.concatenate([res.results[i]["out"] for i in range(N_CORES)], axis=0)
    return out.reshape(BATCH, SEQ, D_OUT), res


def kernel(**inputs) -> np.ndarray:
    out, _ = run(inputs)
    return out


# revision 9
# speedup vs baseline: 1.3553x; 1.0226x over previous
"""TRN2 Bass kernel for CompressedLinearLayer: out = x @ (A @ B.T).T + bias.

Computed low-rank: t = x @ B  (rank 512), out = t @ A.T + bias.
Sharding: data-parallel over the 8192 rows of x (1024 rows per core);
B, A.T, bias replicated. No collectives.

Device layouts (per core), bf16 on the wire for matmul inputs:
  xT   [4096, 1024] bf16  x rows shard, transposed+converted on host
  b    [4096, 512]  bf16  B
  at   [512, 4096]  bf16  A.T
  bias [4096]       f32
  out  [1024, 4096] f32   natural orientation

Per core the 1024 rows are processed in 2 blocks of 512:
  stage1(b): tT[r, m] = sum_k B[k, r] * xT[k, m]   (rank on partitions)
  stage2(b): out[m, d] = sum_r tT[r, m] * AT[r, d] + bias[d]
stage2(0) units are interleaved with stage1(1) groups so the PE never
starves while block-1 x chunks stream in. Inputs stream on the sync
HWDGE ring in 1MB chunks; A.T (ordering-delayed), bias and the output
stores ride the scalar HWDGE ring. Accumulation is fp32 in PSUM.
"""
import numpy as np
import ml_dtypes

import concourse.bacc as bacc
import concourse.mybir as mybir
import concourse.tile as tile
from concourse.tile import add_dep_helper
from concourse.bass_utils import run_bass_kernel_spmd

N_CORES = 8
BATCH, SEQ = 4, 2048
D_IN, D_OUT, RANK = 4096, 4096, 512
ROWS_TOTAL = BATCH * SEQ           # 8192
ROWS = ROWS_TOTAL // N_CORES       # 1024 rows per core

F32 = mybir.dt.float32
BF16 = mybir.dt.bfloat16

KC = D_IN // 128     # 32 contraction chunks, stage 1
KSUB = 4             # k-chunks packed per DMA (0.5MB bf16 transfers)
KB = KC // KSUB      # 4 packed k-groups
RC = RANK // 128     # 4 rank chunks
NBLK = 2             # row blocks per core
BROWS = ROWS // NBLK                 # 512 rows per block
MB2 = BROWS // 128   # 4 row chunks of 128 per block (stage-2 out partitions)
DB2 = D_OUT // 512   # 8 d_out blocks of 512 (stage-2 moving dim)

_compiled = {}


def _build():
    nc = bacc.Bacc("TRN2", target_bir_lowering=False, debug=False)

    xT_d = nc.declare_dram_parameter("xT", [D_IN, ROWS], BF16, isOutput=False)
    b_d = nc.declare_dram_parameter("b", [D_IN, RANK], BF16, isOutput=False)
    at_d = nc.declare_dram_parameter("at", [RANK, D_OUT], BF16, isOutput=False)
    bias_d = nc.declare_dram_parameter("bias", [D_OUT], F32, isOutput=False)
    out_d = nc.declare_dram_parameter("out", [ROWS, D_OUT], F32, isOutput=True)

    with tile.TileContext(nc) as tc:
        with (
            tc.tile_pool(name="wb", bufs=1) as wb,
            tc.tile_pool(name="xp", bufs=4) as xp,
            tc.tile_pool(name="tt", bufs=1) as ttp,
            tc.tile_pool(name="op", bufs=3) as op,
            tc.tile_pool(name="ps1", bufs=4, space="PSUM") as ps1p,
            tc.tile_pool(name="ps2", bufs=4, space="PSUM") as ps2p,
        ):
            bias_bc = wb.tile([128, D_OUT], F32, tag="bias_bc")

            # B resident: 8 tiles [128, 4, 512] bf16 = 0.5MB each
            b_sb = [
                wb.tile([128, KSUB, RANK], BF16, tag=f"b{g}", name=f"b{g}")
                for g in range(KB)
            ]
            # A.T resident: 4 tiles [128, 4096] bf16 (1MB each)
            at_sb = [
                wb.tile([128, D_OUT], BF16, tag=f"at{r}", name=f"at{r}")
                for r in range(RC)
            ]
            # tT per block: 4 tiles [128, 512] bf16 each
            tT = [
                [
                    ttp.tile([128, BROWS], BF16, tag=f"tT{b}_{r}", name=f"tT{b}_{r}")
                    for r in range(RC)
                ]
                for b in range(NBLK)
            ]

            x_dmas = {}

            def stage1_group(b, g, psum1):
                if b == 0:
                    nc.sync.dma_start(
                        b_sb[g][:],
                        b_d[g * KSUB * 128:(g + 1) * KSUB * 128, :].rearrange(
                            "(ks p) r -> p ks r", p=128
                        ),
                    )
                xg = xp.tile([128, KSUB, BROWS], BF16, tag="xk", name=f"x{b}_{g}")
                x_dmas[(b, g)] = nc.sync.dma_start(
                    xg[:],
                    xT_d[
                        g * KSUB * 128:(g + 1) * KSUB * 128,
                        b * BROWS:(b + 1) * BROWS,
                    ].rearrange("(ks p) m -> p ks m", p=128),
                )
                last = g == KB - 1
                if not last:
                    for ks in range(KSUB):
                        k = g * KSUB + ks
                        for mc in range(RC):
                            nc.tensor.matmul(
                                psum1[mc][:],
                                b_sb[g][:, ks, mc * 128:(mc + 1) * 128],
                                xg[:, ks, :],
                                start=(k == 0),
                                stop=False,
                            )
                else:
                    # invert loops so each psum finishes (and can evacuate to
                    # tT on the DVE) while the PE continues with the next mc
                    for mc in range(RC):
                        for ks in range(KSUB):
                            k = g * KSUB + ks
                            nc.tensor.matmul(
                                psum1[mc][:],
                                b_sb[g][:, ks, mc * 128:(mc + 1) * 128],
                                xg[:, ks, :],
                                start=False,
                                stop=(ks == KSUB - 1),
                            )
                        nc.vector.tensor_copy(tT[b][mc][:], psum1[mc][:])

            def stage1_psum(b):
                return [
                    ps1p.tile([128, BROWS], F32, tag="ps1", name=f"ps1_{b}_{i}")
                    for i in range(RC)
                ]

            def load_at_chunk(r, after_dma):
                # scalar HWDGE ring, ordering-delayed so it doesn't starve
                # the block-0 x stream of HBM bandwidth
                at_dma = nc.scalar.dma_start(
                    at_sb[r][:], at_d[r * 128:(r + 1) * 128, :]
                )
                if after_dma is not None:
                    add_dep_helper(
                        at_dma.ins,
                        after_dma.ins,
                        sync=True,
                        reason="delay A.T load behind block-0 x stream",
                    )

            def load_bias():
                nc.scalar.dma_start(bias_bc[0:1, :], bias_d[None, :])
                nc.gpsimd.partition_broadcast(bias_bc[:], bias_bc[0:1, :])

            def stage2_unit(b, rc2, dch):
                row0 = rc2 * 128
                psum2 = [
                    ps2p.tile(
                        [128, 512], F32, tag="ps2",
                        name=f"ps2_{b}_{rc2}_{dch}_{i}",
                    )
                    for i in range(4)
                ]
                for k in range(RC):
                    for dc in range(4):
                        d0 = (dch * 4 + dc) * 512
                        nc.tensor.matmul(
                            psum2[dc][:],
                            tT[b][k][:, row0:row0 + 128],
                            at_sb[k][:, d0:d0 + 512],
                            start=(k == 0),
                            stop=(k == RC - 1),
                        )
                ot = op.tile([128, 2048], F32, tag="ot", name=f"ot{b}_{rc2}_{dch}")
                for dc in range(2):
                    d0 = (dch * 4 + dc * 2) * 512
                    nc.vector.tensor_add(
                        ot[:, dc * 1024:dc * 1024 + 512],
                        psum2[dc * 2][:],
                        bias_bc[:, d0:d0 + 512],
                    )
                    nc.vector.tensor_add(
                        ot[:, dc * 1024 + 512:(dc + 1) * 1024],
                        psum2[dc * 2 + 1][:],
                        bias_bc[:, d0 + 512:d0 + 1024],
                    )
                    nc.scalar.dma_start(
                        out_d[
                            b * BROWS + row0:b * BROWS + row0 + 128,
                            dch * 2048 + dc * 1024:dch * 2048 + (dc + 1) * 1024,
                        ],
                        ot[:, dc * 1024:(dc + 1) * 1024],
                    )

            load_bias()

            # stage1 block 0
            ps_a = stage1_psum(0)
            for g in range(KB):
                stage1_group(0, g, ps_a)
            for r in range(RC):
                load_at_chunk(r, x_dmas[(0, KB - RC + r)])

            # interleave stage2(0) units with stage1(1) groups
            ps_b = stage1_psum(1)
            units = [(0, rc2, dch) for rc2 in range(MB2) for dch in range(DB2 // 4)]
            gi = 0
            for i, u in enumerate(units):
                stage2_unit(*u)
                if gi < KB:
                    stage1_group(1, gi, ps_b)
                    gi += 1
            while gi < KB:
                stage1_group(1, gi, ps_b)
                gi += 1

            for rc2 in range(MB2):
                for dch in range(DB2 // 4):
                    stage2_unit(1, rc2, dch)

    nc.compile()
    return nc


def _get_nc():
    if "nc" not in _compiled:
        _compiled["nc"] = _build()
    return _compiled["nc"]


def run(inputs, trace=False, trace_kwargs=None):
    """Shard, execute on 8 cores, gather. Returns (output, BassKernelResults)."""
    x = np.asarray(inputs["x"], dtype=np.float32)
    A = np.asarray(inputs["A"], dtype=np.float32)
    B = np.asarray(inputs["B"], dtype=np.float32)
    bias = np.asarray(inputs["bias"], dtype=np.float32)

    x_flat = x.reshape(ROWS_TOTAL, D_IN)
    B_bf = B.astype(ml_dtypes.bfloat16)
    AT_bf = np.ascontiguousarray(A.T).astype(ml_dtypes.bfloat16)
    in_maps = []
    for i in range(N_CORES):
        xT_i = np.ascontiguousarray(x_flat[i * ROWS:(i + 1) * ROWS].T).astype(
            ml_dtypes.bfloat16
        )
        in_maps.append({"xT": xT_i, "b": B_bf, "at": AT_bf, "bias": bias})

    nc = _get_nc()
    kwargs = {}
    if trace:
        kwargs["trace"] = True
        kwargs["trace_kwargs"] = trace_kwargs or {}
    res = run_bass_kernel_spmd(nc, in_maps, core_ids=list(range(N_CORES)), **kwargs)

    out = np.concatenate([res.results[i]["out"] for i in range(N_CORES)], axis=0)
    return out.reshape(BATCH, SEQ, D_OUT), res


def kernel(**inputs) -> np.ndarray:
    out, _ = run(inputs)
    return out


# revision 10
# speedup vs baseline: 1.4001x; 1.0330x over previous
"""TRN2 Bass kernel for CompressedLinearLayer: out = x @ (A @ B.T).T + bias.

Computed low-rank: t = x @ B  (rank 512), out = t @ A.T + bias.
Sharding: data-parallel over the 8192 rows of x (1024 rows per core);
B, A.T, bias replicated. No collectives.

Device layouts (per core), bf16 on the wire for matmul inputs:
  xT   [4096, 1024] bf16  x rows shard, transposed+converted on host
  b    [4096, 512]  bf16  B
  at   [512, 4096]  bf16  A.T
  bias [4096]       f32
  out  [1024, 4096] f32   natural orientation

Per core the 1024 rows are processed in 2 blocks of 512:
  stage1(b): tT[r, m] = sum_k B[k, r] * xT[k, m]   (rank on partitions)
  stage2(b): out[m, d] = sum_r tT[r, m] * AT[r, d] + bias[d]
stage2(0) units are interleaved with stage1(1) groups so the PE never
starves while block-1 x chunks stream in. Inputs stream on the sync
HWDGE ring in 1MB chunks; A.T (ordering-delayed), bias and the output
stores ride the scalar HWDGE ring. Accumulation is fp32 in PSUM.
"""
import numpy as np
import ml_dtypes

import concourse.bacc as bacc
import concourse.mybir as mybir
import concourse.tile as tile
from concourse.tile import add_dep_helper
from concourse.bass_utils import run_bass_kernel_spmd

N_CORES = 8
BATCH, SEQ = 4, 2048
D_IN, D_OUT, RANK = 4096, 4096, 512
ROWS_TOTAL = BATCH * SEQ           # 8192
ROWS = ROWS_TOTAL // N_CORES       # 1024 rows per core

F32 = mybir.dt.float32
BF16 = mybir.dt.bfloat16

KC = D_IN // 128     # 32 contraction chunks, stage 1
KSUB = 4             # k-chunks packed per DMA (0.5MB bf16 transfers)
KB = KC // KSUB      # 4 packed k-groups
RC = RANK // 128     # 4 rank chunks
NBLK = 2             # row blocks per core
BROWS = ROWS // NBLK                 # 512 rows per block
MB2 = BROWS // 128   # 4 row chunks of 128 per block (stage-2 out partitions)
DB2 = D_OUT // 512   # 8 d_out blocks of 512 (stage-2 moving dim)

_compiled = {}


def _build():
    nc = bacc.Bacc("TRN2", target_bir_lowering=False, debug=False)

    xT_d = nc.declare_dram_parameter("xT", [D_IN, ROWS], BF16, isOutput=False)
    b_d = nc.declare_dram_parameter("b", [D_IN, RANK], BF16, isOutput=False)
    at_d = nc.declare_dram_parameter("at", [RANK, D_OUT], BF16, isOutput=False)
    bias_d = nc.declare_dram_parameter("bias", [D_OUT], F32, isOutput=False)
    out_d = nc.declare_dram_parameter("out", [ROWS, D_OUT], F32, isOutput=True)

    with tile.TileContext(nc) as tc:
        with (
            tc.tile_pool(name="wb", bufs=1) as wb,
            tc.tile_pool(name="xp", bufs=4) as xp,
            tc.tile_pool(name="tt", bufs=1) as ttp,
            tc.tile_pool(name="op", bufs=3) as op,
            tc.tile_pool(name="ps1", bufs=4, space="PSUM") as ps1p,
            tc.tile_pool(name="ps2", bufs=4, space="PSUM") as ps2p,
        ):
            bias_bc = wb.tile([128, D_OUT], F32, tag="bias_bc")

            # B resident: 8 tiles [128, 4, 512] bf16 = 0.5MB each
            b_sb = [
                wb.tile([128, KSUB, RANK], BF16, tag=f"b{g}", name=f"b{g}")
                for g in range(KB)
            ]
            # A.T resident: 4 tiles [128, 4096] bf16 (1MB each)
            at_sb = [
                wb.tile([128, D_OUT], BF16, tag=f"at{r}", name=f"at{r}")
                for r in range(RC)
            ]
            # tT per block: 4 tiles [128, 512] bf16 each
            tT = [
                [
                    ttp.tile([128, BROWS], BF16, tag=f"tT{b}_{r}", name=f"tT{b}_{r}")
                    for r in range(RC)
                ]
                for b in range(NBLK)
            ]

            x_dmas = {}

            def stage1_group(b, g, psum1):
                split = KSUB if (b == 0 and g == 0) else 1
                if b == 0:
                    for sp in range(split):
                        lo, hi = sp * KSUB // split, (sp + 1) * KSUB // split
                        nc.sync.dma_start(
                            b_sb[g][:, lo:hi, :],
                            b_d[(g * KSUB + lo) * 128:(g * KSUB + hi) * 128, :]
                            .rearrange("(ks p) r -> p ks r", p=128),
                        )
                xg = xp.tile([128, KSUB, BROWS], BF16, tag="xk", name=f"x{b}_{g}")
                for sp in range(split):
                    lo, hi = sp * KSUB // split, (sp + 1) * KSUB // split
                    x_dmas[(b, g)] = nc.sync.dma_start(
                        xg[:, lo:hi, :],
                        xT_d[
                            (g * KSUB + lo) * 128:(g * KSUB + hi) * 128,
                            b * BROWS:(b + 1) * BROWS,
                        ].rearrange("(ks p) m -> p ks m", p=128),
                    )
                last = g == KB - 1
                if not last:
                    for ks in range(KSUB):
                        k = g * KSUB + ks
                        for mc in range(RC):
                            nc.tensor.matmul(
                                psum1[mc][:],
                                b_sb[g][:, ks, mc * 128:(mc + 1) * 128],
                                xg[:, ks, :],
                                start=(k == 0),
                                stop=False,
                            )
                else:
                    # invert loops so each psum finishes (and can evacuate to
                    # tT on the DVE) while the PE continues with the next mc
                    for mc in range(RC):
                        for ks in range(KSUB):
                            k = g * KSUB + ks
                            nc.tensor.matmul(
                                psum1[mc][:],
                                b_sb[g][:, ks, mc * 128:(mc + 1) * 128],
                                xg[:, ks, :],
                                start=False,
                                stop=(ks == KSUB - 1),
                            )
                        nc.vector.tensor_copy(tT[b][mc][:], psum1[mc][:])

            def stage1_psum(b):
                return [
                    ps1p.tile([128, BROWS], F32, tag="ps1", name=f"ps1_{b}_{i}")
                    for i in range(RC)
                ]

            def load_at_chunk(r, after_dma):
                # scalar HWDGE ring, ordering-delayed so it doesn't starve
                # the block-0 x stream of HBM bandwidth
                at_dma = nc.scalar.dma_start(
                    at_sb[r][:], at_d[r * 128:(r + 1) * 128, :]
                )
                if after_dma is not None:
                    add_dep_helper(
                        at_dma.ins,
                        after_dma.ins,
                        sync=True,
                        reason="delay A.T load behind block-0 x stream",
                    )

            def load_bias():
                nc.scalar.dma_start(bias_bc[0:1, :], bias_d[None, :])
                nc.gpsimd.partition_broadcast(bias_bc[:], bias_bc[0:1, :])

            def stage2_unit(b, rc2, dch):
                row0 = rc2 * 128
                psum2 = [
                    ps2p.tile(
                        [128, 512], F32, tag="ps2",
                        name=f"ps2_{b}_{rc2}_{dch}_{i}",
                    )
                    for i in range(4)
                ]
                for k in range(RC):
                    for dc in range(4):
                        d0 = (dch * 4 + dc) * 512
                        nc.tensor.matmul(
                            psum2[dc][:],
                            tT[b][k][:, row0:row0 + 128],
                            at_sb[k][:, d0:d0 + 512],
                            start=(k == 0),
                            stop=(k == RC - 1),
                        )
                ot = op.tile([128, 2048], F32, tag="ot", name=f"ot{b}_{rc2}_{dch}")
                for dc in range(2):
                    d0 = (dch * 4 + dc * 2) * 512
                    nc.vector.tensor_add(
                        ot[:, dc * 1024:dc * 1024 + 512],
                        psum2[dc * 2][:],
                        bias_bc[:, d0:d0 + 512],
                    )
                    nc.vector.tensor_add(
                        ot[:, dc * 1024 + 512:(dc + 1) * 1024],
                        psum2[dc * 2 + 1][:],
                        bias_bc[:, d0 + 512:d0 + 1024],
                    )
                    nc.scalar.dma_start(
                        out_d[
                            b * BROWS + row0:b * BROWS + row0 + 128,
                            dch * 2048 + dc * 1024:dch * 2048 + (dc + 1) * 1024,
                        ],
                        ot[:, dc * 1024:(dc + 1) * 1024],
                    )

            load_bias()

            # stage1 block 0
            ps_a = stage1_psum(0)
            for g in range(KB):
                stage1_group(0, g, ps_a)
            for r in range(RC):
                load_at_chunk(r, x_dmas[(0, KB - RC + r)])

            # interleave stage2(0) units with stage1(1) groups
            ps_b = stage1_psum(1)
            units = [(0, rc2, dch) for rc2 in range(MB2) for dch in range(DB2 // 4)]
            gi = 0
            for i, u in enumerate(units):
                stage2_unit(*u)
                if gi < KB:
                    stage1_group(1, gi, ps_b)
                    gi += 1
            while gi < KB:
                stage1_group(1, gi, ps_b)
                gi += 1

            for rc2 in range(MB2):
                for dch in range(DB2 // 4):
                    stage2_unit(1, rc2, dch)

    nc.compile()
    return nc


def _get_nc():
    if "nc" not in _compiled:
        _compiled["nc"] = _build()
    return _compiled["nc"]


def run(inputs, trace=False, trace_kwargs=None):
    """Shard, execute on 8 cores, gather. Returns (output, BassKernelResults)."""
    x = np.asarray(inputs["x"], dtype=np.float32)
    A = np.asarray(inputs["A"], dtype=np.float32)
    B = np.asarray(inputs["B"], dtype=np.float32)
    bias = np.asarray(inputs["bias"], dtype=np.float32)

    x_flat = x.reshape(ROWS_TOTAL, D_IN)
    B_bf = B.astype(ml_dtypes.bfloat16)
    AT_bf = np.ascontiguousarray(A.T).astype(ml_dtypes.bfloat16)
    in_maps = []
    for i in range(N_CORES):
        xT_i = np.ascontiguousarray(x_flat[i * ROWS:(i + 1) * ROWS].T).astype(
            ml_dtypes.bfloat16
        )
        in_maps.append({"xT": xT_i, "b": B_bf, "at": AT_bf, "bias": bias})

    nc = _get_nc()
    kwargs = {}
    if trace:
        kwargs["trace"] = True
        kwargs["trace_kwargs"] = trace_kwargs or {}
    res = run_bass_kernel_spmd(nc, in_maps, core_ids=list(range(N_CORES)), **kwargs)

    out = np.concatenate([res.results[i]["out"] for i in range(N_CORES)], axis=0)
    return out.reshape(BATCH, SEQ, D_OUT), res


def kernel(**inputs) -> np.ndarray:
    out, _ = run(inputs)
    return out
